# revision 7
# baseline (speedup 1.0000x reference)
"""ChannelAttention TRN2 kernel — channel-major layout.

Math (per token t, head h; hd=16):
  qkv = x @ w_qkv + b_qkv ; q,k,v = split(qkv)
  A[i,j] = softmax_j( scale * q[t,h,i] * k[t,h,j] )
  out[t,h,i] = sum_j A[i,j] v[t,h,j] ;  y = out @ w_proj + b_proj

Tokens are sharded 8 ways (weights replicated, no collectives).

Device layout: channels on SBUF partitions (p = 16*h + c), tokens on the
free dim.  Per T-token tile:
  PE:  qT/kT/vT = W^T @ xT (3 matmuls), per-i masked block-sum matmuls
       that reduce over j into PSUM f32 (num & den), proj matmul.
  ACT: the exp, plus bias-add PSUM->SBUF casts (bias is per-partition
       in this layout, so it rides the activation's bias operand).
  DVE: z = qrep * k (2x bf16), ev = e * v (2x bf16), reciprocal, o = num*rd.
  DMA: q is staged to DRAM and read back replicated 16x along j so the
       outer product runs as a plain elementwise mul; per-i reads engage
       all 128 partitions per descriptor.  DMA work is batched over GRP
       token groups and issuance is split between the SP HWDGE path and
       the idle GpSimd SWDGE path.

The j-reduction runs on the PE via constant 0/1 masks S_i[(h,j),(h',i')] =
(h'==h)&(i'==i): accumulating matmul_i S_i^T @ EV[:, i, :] over i yields
num[(h,i), t] = sum_j E*v in PSUM f32 (same for den with rhs E).
"""

import numpy as np

B, L, C = 4, 16384, 128
H, HD = 8, 16
NCORES = 8
NTOK = B * L
TPC = NTOK // NCORES  # 8192 tokens per core
SCALE = float(C) ** -0.5

T = 512     # tokens per compute tile (psum bank = 512 f32)
GRP = 1024  # tokens per DMA group

_BUILT = None
_LAST_IN_MAPS = None


def _smask_np():
    import ml_dtypes

    S = np.zeros((C, HD, C), np.float32)
    for h in range(H):
        for i in range(HD):
            S[16 * h : 16 * h + 16, i, 16 * h + i] = 1.0
    return S.astype(ml_dtypes.bfloat16)


def _build(repeat=1, tpc=TPC, t_tile=T, grp=GRP):
    import concourse.bass as bass
    from concourse import bacc
    from concourse import mybir
    from concourse.tile import TileContext
    from contextlib import ExitStack, nullcontext

    f32 = mybir.dt.float32
    bf16 = mybir.dt.bfloat16

    nc = bacc.Bacc("TRN2")
    xT = nc.dram_tensor("xT", [C, tpc], bf16, kind="ExternalInput")
    w_qkv = nc.dram_tensor("w_qkv", [C, 3 * C], f32, kind="ExternalInput")
    b_qkv = nc.dram_tensor("b_qkv", [3 * C], f32, kind="ExternalInput")
    w_proj = nc.dram_tensor("w_proj", [C, C], f32, kind="ExternalInput")
    b_proj = nc.dram_tensor("b_proj", [C], f32, kind="ExternalInput")
    outT = nc.dram_tensor("outT", [C, tpc], bf16, kind="ExternalOutput")
    qstage = nc.dram_tensor("qstage", [C, tpc], bf16, kind="Internal")
    smask_d = nc.inline_tensor(_smask_np(), name="smask")

    with TileContext(nc) as tc, ExitStack() as ctx:
        consts = ctx.enter_context(tc.tile_pool(name="consts", bufs=1))
        qkvp = ctx.enter_context(tc.tile_pool(name="qkvp", bufs=2, space="PSUM"))
        ndp = ctx.enter_context(tc.tile_pool(name="ndp", bufs=2, space="PSUM"))
        yp = ctx.enter_context(tc.tile_pool(name="yp", bufs=1, space="PSUM"))
        xpool = ctx.enter_context(tc.tile_pool(name="xpool", bufs=2))
        qbig = ctx.enter_context(tc.tile_pool(name="qbig", bufs=2))
        kvpool = ctx.enter_context(tc.tile_pool(name="kvpool", bufs=4))
        qrpool = ctx.enter_context(tc.tile_pool(name="qrpool", bufs=2))
        zpool = ctx.enter_context(tc.tile_pool(name="zpool", bufs=2))
        epool = ctx.enter_context(tc.tile_pool(name="epool", bufs=2))
        evpool = ctx.enter_context(tc.tile_pool(name="evpool", bufs=2))
        opool = ctx.enter_context(tc.tile_pool(name="opool", bufs=3))
        ybig = ctx.enter_context(tc.tile_pool(name="ybig", bufs=2))

        # ---- constants ----
        wqkv_bf = consts.tile([C, 3 * C], bf16)
        wp_bf = consts.tile([C, C], bf16)
        with tc.tile_pool(name="wstage", bufs=1) as wstage:
            wqkv_f = wstage.tile([C, 3 * C], f32)
            nc.sync.dma_start(out=wqkv_f, in_=w_qkv[:, :])
            nc.vector.tensor_copy(wqkv_bf[:], wqkv_f[:])
            wp_f = wstage.tile([C, C], f32)
            nc.sync.dma_start(out=wp_f, in_=w_proj[:, :])
            nc.vector.tensor_copy(wp_bf[:], wp_f[:])

        bq = consts.tile([C, 1], f32, tag="bq")
        bk = consts.tile([C, 1], f32, tag="bk")
        bv = consts.tile([C, 1], f32, tag="bv")
        bp = consts.tile([C, 1], f32, tag="bp")
        for g, bt in enumerate((bq, bk, bv)):
            nc.sync.dma_start(out=bt, in_=b_qkv[g * C : (g + 1) * C].unsqueeze(1))
        nc.sync.dma_start(out=bp, in_=b_proj[:].unsqueeze(1))

        smask = consts.tile([C, HD, C], bf16)
        nc.sync.dma_start(out=smask, in_=smask_d[:, :, :])

        if repeat < 0:
            nrep, loop_n = -repeat, 1
        else:
            nrep, loop_n = 1, repeat
        rep_ctx = tc.For_i(0, loop_n, 1) if loop_n > 1 else nullcontext()
        with rep_ctx:
          for _ in range(nrep):
            _emit(
                nc, mybir, tpc, t_tile, grp,
                qkvp, ndp, yp, xpool, qbig, kvpool, qrpool, zpool, epool,
                evpool, opool, ybig,
                wqkv_bf, wp_bf, bq, bk, bv, bp, smask, xT, qstage, outT,
            )

    nc.compile()
    return nc


def _emit(
    nc, mybir, tpc, T, GRP,
    qkvp, ndp, yp, xpool, qbig, kvpool, qrpool, zpool, epool, evpool,
    opool, ybig,
    wqkv_bf, wp_bf, bq, bk, bv, bp, smask, xT, qstage, outT,
):
    f32 = mybir.dt.float32
    bf16 = mybir.dt.bfloat16
    Ident = mybir.ActivationFunctionType.Identity
    Exp = mybir.ActivationFunctionType.Exp
    NG = tpc // GRP
    SUB = GRP // T

    for g in range(NG):
        t0 = g * GRP
        x_big = xpool.tile([C, GRP], bf16, tag="x")
        nc.sync.dma_start(out=x_big, in_=xT[:, t0 : t0 + GRP])

        q_big = qbig.tile([C, GRP], bf16, tag="q")
        kv = []  # (k_sb, v_sb) per sub-tile
        for s in range(SUB):
            xs = x_big[:, s * T : (s + 1) * T]
            k_sb = kvpool.tile([C, T], bf16, tag="k")
            v_sb = kvpool.tile([C, T], bf16, tag="v")
            kv.append((k_sb, v_sb))
            dests = (q_big[:, s * T : (s + 1) * T], k_sb[:], v_sb[:])
            for gq, (dst, bias) in enumerate(zip(dests, (bq, bk, bv))):
                ps = qkvp.tile([C, T], f32, tag="ps")
                nc.tensor.matmul(
                    out=ps[:],
                    lhsT=wqkv_bf[:, gq * C : (gq + 1) * C],
                    rhs=xs,
                    start=True,
                    stop=True,
                )
                nc.scalar.activation(dst, ps[:], Ident, bias=bias[:])

        # stage q to DRAM; read back replicated 16x along j:
        # qrep[p=(h,j), i, t] = q[(h,i), t]
        nc.sync.dma_start(out=qstage[:, t0 : t0 + GRP], in_=q_big[:])
        HH = HD // 2
        qrep_a = qrpool.tile([C, HH, GRP], bf16, tag="qrep_a")
        qrep_b = qrpool.tile([C, HH, GRP], bf16, tag="qrep_b")
        qv = qstage[:, t0 : t0 + GRP].rearrange("(h i) t -> h i t", i=HD)
        for i in range(HD):
            src = qv[:, i, :].unsqueeze(1).broadcast_to((H, HD, GRP))
            eng = nc.sync if i % 2 == 0 else nc.gpsimd
            dst = qrep_a if i < HH else qrep_b
            eng.dma_start(out=dst[:, i % HH, :], in_=src)

        y_big = ybig.tile([C, GRP], bf16, tag="y")
        for s in range(SUB):
            k_sb, v_sb = kv[s]
            k_bc = k_sb[:].unsqueeze(1).broadcast_to((C, HH, T))
            v_bc = v_sb[:].unsqueeze(1).broadcast_to((C, HH, T))
            num = ndp.tile([C, T], f32, tag="num")
            den = ndp.tile([C, T], f32, tag="den")
            for half, qr in enumerate((qrep_a, qrep_b)):
                z = zpool.tile([C, HH, T], bf16, tag=f"z{half}")
                nc.vector.tensor_mul(z[:], qr[:, :, s * T : (s + 1) * T], k_bc)
                e = epool.tile([C, HH, T], bf16, tag=f"e{half}")
                nc.scalar.activation(e[:], z[:], Exp, scale=SCALE)
                ev = evpool.tile([C, HH, T], bf16, tag=f"ev{half}")
                nc.vector.tensor_mul(ev[:], e[:], v_bc)
                for ii in range(HH):
                    i = half * HH + ii
                    nc.tensor.matmul(
                        out=den[:],
                        lhsT=smask[:, i, :],
                        rhs=e[:, ii, :],
                        start=(i == 0),
                        stop=(i == HD - 1),
                    )
                    nc.tensor.matmul(
                        out=num[:],
                        lhsT=smask[:, i, :],
                        rhs=ev[:, ii, :],
                        start=(i == 0),
                        stop=(i == HD - 1),
                    )

            rd = opool.tile([C, T], f32, tag="rd")
            nc.vector.reciprocal_approx_fast(out=rd[:], in_=den[:])
            oT = opool.tile([C, T], bf16, tag="oT")
            nc.vector.tensor_mul(oT[:], num[:], rd[:])

            y_ps = yp.tile([C, T], f32)
            nc.tensor.matmul(
                out=y_ps[:], lhsT=wp_bf[:], rhs=oT[:], start=True, stop=True
            )
            nc.scalar.activation(
                y_big[:, s * T : (s + 1) * T], y_ps[:], Ident, bias=bp[:]
            )

        nc.gpsimd.dma_start(out=outT[:, t0 : t0 + GRP], in_=y_big[:])


def kernel(x, w_qkv, b_qkv, w_proj, b_proj):
    from concourse import bass_utils

    global _BUILT
    if _BUILT is None:
        _BUILT = _build()
    nc = _BUILT

    import ml_dtypes

    xf = np.asarray(x, np.float32).reshape(NTOK, C)
    w_qkv = np.ascontiguousarray(np.asarray(w_qkv, np.float32))
    b_qkv = np.ascontiguousarray(np.asarray(b_qkv, np.float32))
    w_proj = np.ascontiguousarray(np.asarray(w_proj, np.float32))
    b_proj = np.ascontiguousarray(np.asarray(b_proj, np.float32))

    in_maps = []
    for i in range(NCORES):
        shard = xf[i * TPC : (i + 1) * TPC]
        in_maps.append(
            {
                "xT": np.ascontiguousarray(shard.T).astype(ml_dtypes.bfloat16),
                "w_qkv": w_qkv,
                "b_qkv": b_qkv,
                "w_proj": w_proj,
                "b_proj": b_proj,
            }
        )

    global _LAST_IN_MAPS
    _LAST_IN_MAPS = in_maps
    res = bass_utils.run_bass_kernel_spmd(nc, in_maps, core_ids=list(range(NCORES)))
    y = np.concatenate(
        [
            np.asarray(res.results[i]["outT"]).astype(np.float32).T
            for i in range(NCORES)
        ],
        axis=0,
    )
    return y.reshape(B, L, C)
